# revision 3
# baseline (speedup 1.0000x reference)
"""Trainium2 Bass kernel for nn_AttentionHead (B=32, C=256, H=W=32), v2.

Reference computation (per batch b):
    xs = x[b].reshape(C, S).T                     # [S, C], S = H*W = 1024
    q = xs @ wq.T + bq ; k = xs @ wk.T + bk ; v = xs @ wv.T + bv
    attn = softmax(q @ k.T / sqrt(C), axis=-1)    # [S, S]
    out[b] = silu(attn @ v).T.reshape(C, H, W)

Sharding: data-parallel over B across 8 cores (4 batches/core); weights
replicated.

v2 algebraic restructure (vs v1's separate q/k projections):
    logit[s,t] = (wq xs_s + bq)·(wk xs_t + bk)
               = xs_s'M xs_t + (wk'bq)·xs_t + [terms const per s]
with M = wq'wk.  The per-s terms multiply every e[t,s] by a constant
g(s) which cancels exactly in softmax normalization, so they are
DROPPED.  The device computes
    z = M' xs            (one C*C projection instead of two)
    scoresT[t,s] = xs_t . z_s   (contract over c)
    e = exp(scoresT/16 + bias_t),  bias_t = (wk'bq)·xs_t / 16
and bias_t rides for free in column C+1 of the extended v matmul
(v_ext = xs'@[wv.T | 0 | u/16] + [bv | 2 | 0]), whose column C is the
denominator trick: attn@v_ext yields 2*denom in column C.
No max-subtraction: |exp args| <= ~9, safe in fp32.

Tail per output tile (out = silu(po/denom), via tanh since Exp/Tanh
share one ACT table set):  rec = 1/(2 denom);
    th = Tanh(po * rec)          (ACT, per-partition scale AP)
    t2 = (th + 1) * rec          (DVE two-op tensor_scalar)
    out = po * t2                (= h(1+tanh h), h = po*rec)
All matmul operands are bf16 (HW-measured: ~1 row/cycle @2.4GHz for
free-dim >= 512 regardless of dtype, so bf16 costs no PE time, halves
DMA/SBUF, and enables FWL weight loads; fp8+DoubleRow would halve PE
time but fails the 2e-2 error budget).  PSUM accumulation stays fp32.
x prefetch is software-pipelined: batch it+1's x DMAs issue before
batch it's out stores so they are not stuck behind them in the Sync
DGE ring.  The [B, S, C] device output is transposed on the host.
"""

import numpy as np

import concourse.tile as tile
from concourse import bacc, mybir
from concourse.bass_utils import run_bass_kernel_spmd

F32 = mybir.dt.float32
F32R = mybir.dt.float32r
BF16 = mybir.dt.bfloat16
AF = mybir.ActivationFunctionType
ALU = mybir.AluOpType

B, C, H, W = 32, 256, 32, 32
S = H * W              # 1024
N_CORES = 8
BPC = B // N_CORES     # 4 batches per core
CT = C // 128          # 2 contraction tiles
DT = C // 128          # 2 output-channel tiles
TT = S // 128          # 8 key/query row tiles
NS = S // 512          # 2 512-wide column chunks
SPH = TT // NS         # 4 s-tiles per n-chunk
SCALE = 1.0 / 16.0     # 1/sqrt(C)


def _build_attention_core(iters=1, loop_n=None):
    nc = bacc.Bacc("TRN2", debug=False)

    x_d = nc.dram_tensor("x", [BPC, C, S], BF16, kind="ExternalInput")
    m_d = nc.dram_tensor("m_t", [C, C], BF16, kind="ExternalInput")
    wv_d = nc.dram_tensor("wv_e", [C, C + 2], BF16, kind="ExternalInput")
    bv_d = nc.dram_tensor("bv_e", [1, C + 2], BF16, kind="ExternalInput")
    out_d = nc.dram_tensor("out", [BPC, S, C], F32, kind="ExternalOutput")

    with tile.TileContext(nc) as tc:
        with (
            tc.tile_pool(name="consts", bufs=1) as consts,
            tc.tile_pool(name="xp", bufs=2) as xp,
            tc.tile_pool(name="zp", bufs=2) as zp,
            tc.tile_pool(name="vp", bufs=2) as vp,
            tc.tile_pool(name="ep", bufs=2) as ep,
            tc.tile_pool(name="op", bufs=4) as op,
            tc.tile_pool(name="ps_vo", bufs=4, space="PSUM") as ps_vo,
            tc.tile_pool(name="ps_s", bufs=2, space="PSUM") as ps_s,
        ):
            def load_x_into(x_sb, b):
                for n in range(NS):
                    for ct in range(CT):
                        nc.sync.dma_start(
                            out=x_sb[:, ct, n * 512:(n + 1) * 512],
                            in_=x_d.ap()[b, ct * 128:(ct + 1) * 128,
                                         n * 512:(n + 1) * 512],
                        )

            def load_x(b):
                x_sb = xp.tile([128, CT, S], BF16, name=f"x_{b}", tag="x")
                load_x_into(x_sb, b)
                return x_sb

            # DMA emission order tracks the first matmuls' needs: m and the
            # two n=0 chunks of batch-0 x unblock the first z projection.
            m_sb = consts.tile([128, CT, C], BF16)
            wv_sb = consts.tile([128, CT, C + 2], BF16)
            bv_sb = consts.tile([128, C + 2], BF16)
            x_pref = xp.tile([128, CT, S], BF16, name="x_pref", tag="x")
            # warm the PE HAM clock while the first DMAs are in flight:
            # tiny matmuls on a zeroed scratch tile, results unused
            warm_sb = consts.tile([1, 128], F32)
            nc.vector.memset(warm_sb, 0.0)
            warm_ps = ps_s.tile([128, NS, 512], F32, name="warm_ps", tag="pss")
            for _ in range(8):
                nc.tensor.matmul(
                    warm_ps[0:1, 0, 0:128], warm_sb[0:1, 0:1], warm_sb[0:1, :],
                    start=True, stop=True)
            nc.sync.dma_start(out=m_sb, in_=m_d.ap().rearrange("(ct p) d -> p ct d", p=128))
            for ct in range(CT):
                nc.sync.dma_start(
                    out=x_pref[:, ct, 0:512], in_=x_d.ap()[0, ct * 128:(ct + 1) * 128, 0:512],
                )
            nc.sync.dma_start(out=wv_sb, in_=wv_d.ap().rearrange("(ct p) d -> p ct d", p=128))
            for ct in range(CT):
                nc.sync.dma_start(
                    out=x_pref[:, ct, 512:1024], in_=x_d.ap()[0, ct * 128:(ct + 1) * 128, 512:1024],
                )
            nc.sync.dma_start(out=bv_sb, in_=bv_d.ap().to_broadcast([128, C + 2]))

            import contextlib
            loop_cm = tc.For_i(0, loop_n, 1) if loop_n else contextlib.nullcontext()
            with loop_cm:
              # software-pipelined x prefetch: batch it+1's x DMAs are issued
              # BEFORE batch it's attn@V/output section, so they queue ahead
              # of the (late-firing) out stores on the Sync DGE ring.  Batch
              # 0 always lives in x_pref; in loop mode it is re-loaded for
              # the next For_i iteration during batch 3's section.
              x_tiles = {0: x_pref}
              for it in range(BPC * iters):
                b = it % BPC
                x_sb = x_tiles.pop(it, None)
                if x_sb is None:
                    x_sb = load_x(b)

                # ---- z[c', s] = M-slices @ x  (no bias) ----
                z_sb = zp.tile([128, DT, S], BF16, name=f"z_{b}", tag="z")
                for dt in range(DT):
                    psz = ps_s.tile([128, NS, 512], F32, name="psz", tag="pss")
                    for n in range(NS):
                        for ct in range(CT):
                            nc.tensor.matmul(
                                psz[:, n, :],
                                m_sb[:, ct, dt * 128:(dt + 1) * 128],
                                x_sb[:, ct, n * 512:(n + 1) * 512],
                                start=(ct == 0),
                                stop=(ct == CT - 1),
                            )
                    for n in range(NS):
                        nc.vector.tensor_scalar_mul(
                            z_sb[:, dt, n * 512:(n + 1) * 512],
                            psz[:, n, :],
                            1.0,
                        )

                # ---- v_ext[t, d'] = x-slices.T @ wv_e, + bv (broadcast) ----
                # columns: [0:C] = v + bv, [C] = 2 (denom), [C+1] = exp bias_t
                v_sb = vp.tile([128, TT, C + 2], BF16, name=f"v_{b}", tag="v")
                for tt in range(TT):
                    pv = ps_vo.tile([128, C + 2], F32, name="pv", tag="vo")
                    for ct in range(CT):
                        nc.tensor.matmul(
                            pv,
                            x_sb[:, ct, tt * 128:(tt + 1) * 128],
                            wv_sb[:, ct, :],
                            start=(ct == 0),
                            stop=(ct == CT - 1),
                        )
                    nc.vector.tensor_tensor(
                        v_sb[:, tt, :], pv, bv_sb, op=ALU.add,
                    )

                # ---- per tt: scoresT chunk then exp with per-t bias ----
                e_sb = ep.tile([128, TT, S], BF16, name=f"e_{b}", tag="e")
                for tt in range(TT):
                    pss = ps_s.tile([128, NS, 512], F32, name="pss", tag="pss")
                    for n in range(NS):
                        for ct in range(CT):
                            nc.tensor.matmul(
                                pss[:, n, :],
                                x_sb[:, ct, tt * 128:(tt + 1) * 128],
                                z_sb[:, ct, n * 512:(n + 1) * 512],
                                start=(ct == 0),
                                stop=(ct == CT - 1),
                            )
                    nc.scalar.activation(
                        e_sb[:, tt, :], pss.rearrange("p n f -> p (n f)"), AF.Exp,
                        scale=SCALE,
                        bias=v_sb[:, tt, C + 1:C + 2],
                    )

                # prefetch the next batch's x now, ahead of this batch's out
                # stores in the Sync DGE ring (see note above the loop)
                nxt = it + 1
                if nxt < BPC * iters:
                    x_tiles[nxt] = load_x(nxt % BPC)
                elif loop_n:
                    load_x_into(x_pref, 0)

                # ---- per st: attn@v_ext then silu tail ----
                for st in range(TT):
                    po = ps_vo.tile([128, C + 2], F32, name="po", tag="vo")
                    for tt in range(TT):
                        nc.tensor.matmul(
                            po,
                            e_sb[:, tt, st * 128:(st + 1) * 128],
                            v_sb[:, tt, :],
                            start=(tt == 0),
                            stop=(tt == TT - 1),
                        )
                    rec = op.tile([128, 1], F32, name="rec", tag="rec")
                    nc.vector.reciprocal(rec, po[:, C:C + 1])
                    th = op.tile([128, C], F32, name="th", tag="th")
                    nc.scalar.activation(th, po[:, :C], AF.Tanh, scale=rec)
                    last = it == BPC * iters - 1
                    eng = nc.vector if (last and st % 2 == 1) else nc.gpsimd
                    t2 = op.tile([128, C], F32, name="t2", tag="t2")
                    eng.tensor_scalar(
                        t2, th, 1.0, rec, op0=ALU.add, op1=ALU.mult,
                    )
                    o_sb = op.tile([128, C], F32, name="o_sb", tag="o")
                    nc.vector.tensor_tensor(o_sb, po[:, :C], t2, op=ALU.mult)
                    nc.sync.dma_start(
                        out=out_d.ap()[b, st * 128:(st + 1) * 128, :], in_=o_sb,
                    )

    nc.compile()
    return nc


_NC_CACHE = None


def _get_nc():
    global _NC_CACHE
    if _NC_CACHE is None:
        _NC_CACHE = _build_attention_core()
    return _NC_CACHE


def _make_in_maps(x, wq, bq, wk, bk, wv, bv):
    import ml_dtypes
    x = np.ascontiguousarray(x, dtype=np.float32).reshape(B, C, S)
    x = x.astype(ml_dtypes.bfloat16)
    m_t = np.ascontiguousarray(
        (wq.T.astype(np.float64) @ wk.astype(np.float64)).astype(ml_dtypes.bfloat16))
    u16 = (wk.T.astype(np.float64) @ bq.astype(np.float64)).astype(np.float32) / 16.0
    wv_e = np.zeros((C, C + 2), dtype=np.float32)
    wv_e[:, :C] = wv.T
    wv_e[:, C + 1] = u16
    bv_e = np.zeros((1, C + 2), dtype=np.float32)
    bv_e[0, :C] = bv
    bv_e[0, C] = 2.0
    shared = {"m_t": m_t, "wv_e": wv_e.astype(ml_dtypes.bfloat16),
              "bv_e": bv_e.astype(ml_dtypes.bfloat16)}
    return [
        {"x": x[i * BPC:(i + 1) * BPC], **shared} for i in range(N_CORES)
    ]


def kernel(x, wq, bq, wk, bk, wv, bv, _trace=False):
    nc = _get_nc()
    in_maps = _make_in_maps(
        np.asarray(x), np.asarray(wq), np.asarray(bq), np.asarray(wk),
        np.asarray(bk), np.asarray(wv), np.asarray(bv),
    )
    try:
        res = run_bass_kernel_spmd(nc, in_maps, list(range(N_CORES)), trace=_trace)
    except Exception:
        # the axon-tunneled devices occasionally report a transient
        # NRT_EXEC_UNIT_UNRECOVERABLE right after another process exited;
        # one delayed retry has always recovered in practice
        import time
        time.sleep(15)
        res = run_bass_kernel_spmd(nc, in_maps, list(range(N_CORES)), trace=_trace)
    out = np.concatenate([res.results[i]["out"] for i in range(N_CORES)], axis=0)
    out = out.transpose(0, 2, 1).reshape(B, C, H, W)
    if _trace:
        return np.ascontiguousarray(out, dtype=np.float32), res
    return np.ascontiguousarray(out, dtype=np.float32)
